# revision 52
# baseline (speedup 1.0000x reference)
"""Trainium2 Bass kernel for the Chambolle-Pock-style primal/dual stencil loop.

Math (per image, H=W=1024, EPS=0.5, TAU=0.5, 10 iterations):
    u = sigmoid(o/EPS); q = 0
    repeat 10x:
        q  = relu(q - TAU*(vf1*Dy(u) + vf0*Dx(u)))   # forward diffs, zero pad
        Tq = BDy(vf1*q) + BDx(vf0*q)                  # backward diffs, zero pad
        u  = sigmoid((o - Tq)/EPS)
    return (o - Tq)/EPS

Rescaled (qh = 2*sqrt(2)*q, g = vf/sqrt(2), o2 = 2*o, t = tanh(s/2) with
s = 2(o - Tq); u-padding 0 becomes t-padding -1) and with every y-shifted
product rewritten through a host-preshifted field (gs = g0+g1, g1d(y) =
g1(y-1)) so each product depends on exactly one tanh/relu quarter:
    K = g1d*t; H = g1*qh                  # then B(y)=g1(y)*t(y+1) = K(y+1)
    dual:   qh = relu(qh + gs*t - K(y+1) - g0*t(x+1))
    primal: s  = o2 - gs*qh + H(y-1) + (g0*qh)(x-1)
    t = tanh(s/2); output = s of the last iteration.

State and products are fp16 (DVE tensor_tensor runs at 2 elem/cyc for packed
2-byte operands; bf16 fails the 2e-2 gate because the relu makes isolated
pixels chaotic under per-step rounding noise — measured rel-L2 ~1e-2 for
fp16 vs ~3e-2 for bf16).  The primal accumulation happens in fp32 PSUM.

Engine split (four compute engines + DMA queues, all busy):
  - DVE: the A/K/H/D products, one quarter of F', and quarter q1's dual
    accumulations, all 4B-aligned fp16 tensor_tensor at 2 elem/cycle.
  - GPSIMD: the 2B-misaligned product C~ = g0*t(x+1) (GPSIMD is
    alignment-indifferent) and one quarter of F' = g0*qh.
  - PE (tensor engine): both chains as accumulating +-identity matmuls
    into fp32 PSUM, one 512-column matmul per PSUM bank:
    dual  psum = I*qh + I*A - I*K(y+1) - I*C~   (3 of 4 quarters)
    primal psum = I*o2 - I*(gs*qh) + I*F'(x-1) + I*H(y-1),
    the x-1 shift expressed directly via column-offset PSUM writes (s(0)
    keeps no F term).  Group heads (qh / o2) have no in-iteration deps so
    the PE never stalls at a bank's start matmul.
  - Act: relu and tanh (PSUM -> SBUF fp16), one quarter at a time; on the
    last iteration it instead copies s from PSUM for the output DMA.
  - DMA: the K/H boundary rows move between partitions via tiny SBUF->SBUF
    copies whose consumers sit half an iteration away.

Layout: image row y = 8*p + i -> partition p (0..127), plane i (0..7).
Everything operates at quarter (2-plane) granularity in fixed order
(1,2,3,0); per iteration the DVE does ~45 plane-passes at ~0.56us, GPSIMD
10 at 2.05us, PE 112 x 512-col matmuls (~24us), Act 16 quarter-activations.
Working set (~196KB/partition) is SBUF resident: HBM traffic is one 8MB
fp16 load + 2MB store per core.

Sharding: pure data parallel, one image per NeuronCore (B=8 over 8 cores),
g-fields broadcast.
"""

import numpy as np

import concourse.bacc as bacc
import concourse.mybir as mybir
from concourse.tile import TileContext
from concourse import bass_utils

F16 = mybir.dt.float16
F32 = mybir.dt.float32
AF = mybir.ActivationFunctionType

B, H, W = 8, 1024, 1024
P = 128          # SBUF partitions
NP = H // P      # planes per partition = 8
WG = W + 2       # t-plane width incl. guard column (even, keeps 4B align)
MAXITER = 10
QORD = ((2, 4), (4, 6), (6, 8), (0, 2))   # quarter order 1,2,3,0
BK = 512         # PSUM bank = 512 fp32 = one matmul's max output

_CACHE = {}
LAST_RESULTS = None  # BassKernelResults of the most recent run (for test.py)


def _build(reps=1):
    """Build the Bass program.  reps>1 repeats the whole computation (state
    re-initialized each rep, same output) — used only for wall-clock timing
    of the HW kernel when no NTFF profiling is available."""
    nc = bacc.Bacc("TRN2", target_bir_lowering=False, debug=False)

    o2_d = nc.dram_tensor("o2", [H, W], F16, kind="ExternalInput").ap()
    g0_d = nc.dram_tensor("g0", [H, W], F16, kind="ExternalInput").ap()
    gs_d = nc.dram_tensor("gs", [H, W], F16, kind="ExternalInput").ap()
    g1d_d = nc.dram_tensor("g1d", [H, W], F16, kind="ExternalInput").ap()
    km_d = nc.dram_tensor("km", [1, W], F16, kind="ExternalInput").ap()
    id_d = nc.dram_tensor("ident", [P, P], F16, kind="ExternalInput").ap()
    nid_d = nc.dram_tensor("nident", [P, P], F16, kind="ExternalInput").ap()
    out_d = nc.dram_tensor("out", [H, W], F16, kind="ExternalOutput").ap()

    # (H, W) -> (p, i, x) with y = 8*p + i
    def vu(ap):
        return ap.rearrange("(p i) x -> p i x", i=NP)

    v = nc.vector
    gp = nc.gpsimd
    act = nc.scalar
    pe = nc.tensor

    with TileContext(nc) as tc:
        with tc.tile_pool(name="main", bufs=1) as pool:
            o2t = pool.tile([P, NP, W], F16)
            g0t = pool.tile([P, NP, W], F16)
            g1t = pool.tile([P, NP, W], F16)
            gst = pool.tile([P, NP, W], F16)
            g1dt = pool.tile([P, NP, W], F16)
            # t state: planes 0..7, col W = -1 guard for x+1 reads
            sut = pool.tile([P, NP, WG], F16)
            qht = pool.tile([P, NP, W], F16)
            tA = pool.tile([P, NP, W], F16)
            # K = g1d*t at planes 0..7; plane 8 = K[8p+8] boundary
            # (partition 127: constant -g1[1023] = K at the t=-1 pad row)
            tK = pool.tile([P, NP + 1, W], F16)
            # H = g1*qh at planes 1..8; plane 0 = H[8p-1] boundary
            # (partition 0: zero pad)
            tH = pool.tile([P, NP + 1, W], F16)
            tC = pool.tile([P, NP, W], F16)
            tF = pool.tile([P, NP, W], F16)
            idt = pool.tile([P, P], F16)
            nidt = pool.tile([P, P], F16)

            def t_(lo, hi):     # t rows 8p+lo..8p+hi-1
                return sut[:, lo:hi, 0:W]

            def trt(lo, hi):    # t(x+1) (col W = -1 guard)
                return sut[:, lo:hi, 1 : W + 1]

            def bv(lo, hi):     # B(y) = K(y+1) (plane 8 = boundary)
                return tK[:, lo + 1 : hi + 1, :]

            def qh_(lo, hi):
                return qht[:, lo:hi, :]

            def sl(tile, lo, hi):
                return tile[:, lo:hi, :]

            # --- setup ---
            # Only guard regions need init: everything else is written
            # before its first read.  Loads are HBM-bandwidth serial
            # (~1.6us per field quarter), so interleave in need order;
            # g1 = gs - g0 is derived on-chip to cut the load volume.
            v.memset(sut[:, :, W:WG], -1.0)        # x+1 guard column = -1
            v.memset(tH[0:1, 0, :], 0.0)           # H[-1] pad row = 0
            nc.sync.dma_start(out=tK[P - 1 : P, NP, :], in_=km_d)
            nc.sync.dma_start(out=idt[:, :], in_=id_d)
            nc.sync.dma_start(out=nidt[:, :], in_=nid_d)
            for lo, hi in QORD:
                nc.sync.dma_start(
                    out=o2t[:, lo:hi, :], in_=vu(o2_d)[:, lo:hi, :]
                )
                nc.sync.dma_start(
                    out=g0t[:, lo:hi, :], in_=vu(g0_d)[:, lo:hi, :]
                )
                nc.sync.dma_start(
                    out=gst[:, lo:hi, :], in_=vu(gs_d)[:, lo:hi, :]
                )
                nc.sync.dma_start(
                    out=g1dt[:, lo:hi, :], in_=vu(g1d_d)[:, lo:hi, :]
                )

            def dma_kshift():
                # tK[p, 8] = K[8p+8] = tK[p+1, 0]; partition 127 keeps km
                nc.sync.dma_start(
                    out=tK[0 : P - 1, NP, :], in_=tK[1:P, 0, :]
                )

            def dma_hshift():
                # tH[p, 0] = H[8p-1] = tH[p-1, 8]; partition 0 stays 0
                nc.sync.dma_start(
                    out=tH[1:P, 0, :], in_=tH[0 : P - 1, NP, :]
                )

            with tc.tile_pool(name="ps", bufs=2, space="PSUM") as pp:

                def alloc_ps():
                    # single allocation site -> one rotating pair of 4-bank
                    # PSUM buffers shared by the dual and primal chains
                    return pp.tile([P, 2, W], F32, name="ps")

                for _rep in range(reps):
                    for lo, hi in QORD:
                        act.activation(
                            t_(lo, hi), sl(o2t, lo, hi), AF.Tanh, scale=0.5
                        )

                    for it in range(MAXITER):
                        first = it == 0
                        last = it == MAXITER - 1
                        # --- dual products (each needs one tanh quarter) ---
                        # GPSIMD: C~ = g0*t(x+1) (misaligned read is free here)
                        for lo, hi in QORD:
                            gp.tensor_mul(
                                sl(tC, lo, hi), sl(g0t, lo, hi), trt(lo, hi)
                            )
                        # DVE: A = gs*t, K = g1d*t; g1 = gs - g0 (iter 0 only)
                        for lo, hi in QORD:
                            v.tensor_mul(sl(tA, lo, hi), sl(gst, lo, hi), t_(lo, hi))
                            v.tensor_mul(sl(tK, lo, hi), sl(g1dt, lo, hi), t_(lo, hi))
                            if first:
                                v.tensor_sub(
                                    sl(g1t, lo, hi), sl(gst, lo, hi), sl(g0t, lo, hi)
                                )
                        dma_kshift()
                        # --- dual chain: qh = relu(qh + A - K(y+1) - C~).
                        # Quarter q0 (processed last) runs as plain DVE
                        # accumulations — DVE has slack right after its
                        # product phase and q0's B-view needs no boundary
                        # row; the other quarters run on PE as +-identity
                        # matmuls into fp32 PSUM with relu reading PSUM on
                        # Act.  The PE group head (qh) has no in-iteration
                        # deps, so the PE never stalls at a start matmul.
                        for lo, hi in QORD:
                            if lo == 2:
                                if first:
                                    v.tensor_sub(
                                        qh_(lo, hi), sl(tA, lo, hi), bv(lo, hi)
                                    )
                                else:
                                    v.tensor_add(
                                        qh_(lo, hi), qh_(lo, hi), sl(tA, lo, hi)
                                    )
                                    v.tensor_sub(
                                        qh_(lo, hi), qh_(lo, hi), bv(lo, hi)
                                    )
                                v.tensor_sub(qh_(lo, hi), qh_(lo, hi), sl(tC, lo, hi))
                                act.activation(qh_(lo, hi), qh_(lo, hi), AF.Relu)
                                continue
                            ps = alloc_ps()
                            srcs = [] if first else [(idt, qht, lo)]
                            srcs += [(idt, tA, lo), (nidt, tK, lo + 1), (nidt, tC, lo)]
                            for i_s, (w, tile, plo) in enumerate(srcs):
                                for p in range(2):
                                    for c in (0, BK):
                                        pe.matmul(
                                            ps[:, p, c : c + BK], w[:, :],
                                            tile[:, plo + p, c : c + BK],
                                            start=(i_s == 0),
                                            stop=(i_s == len(srcs) - 1),
                                        )
                            act.activation(qh_(lo, hi), ps[:, :, :], AF.Relu)
                        # --- primal products (each needs one relu quarter);
                        # they must ALL precede the PE loop in program order
                        # because the E-view of quarter q reads H rows from
                        # quarter q-1 (including the wrap onto q0, last) ---
                        for lo, hi in QORD:
                            # F' = g0*qh: quarters q1/q2 on GPSIMD, rest DVE
                            if lo in (2,):
                                gp.tensor_mul(
                                    sl(tF, lo, hi), sl(g0t, lo, hi), qh_(lo, hi)
                                )
                            else:
                                v.tensor_mul(
                                    sl(tF, lo, hi), sl(g0t, lo, hi), qh_(lo, hi)
                                )
                            # H = g1*qh (planes 1..8 of tH), D = gs*qh
                            v.tensor_mul(
                                tH[:, lo + 1 : hi + 1, :], sl(g1t, lo, hi),
                                qh_(lo, hi),
                            )
                            if (lo, hi) == (6, 8):
                                dma_hshift()
                            v.tensor_mul(sl(tA, lo, hi), sl(gst, lo, hi), qh_(lo, hi))
                        # --- primal accumulation on PE: s = o2 - D + F'(x-1)
                        # + H(y-1), one matmul per PSUM bank, E-view last so
                        # only the group tail waits on the wrap H quarter;
                        # accumulation groups are tracked per 2KB bank, so
                        # each bank's final matmul carries stop=True ---
                        for lo, hi in QORD:
                            ps = alloc_ps()
                            for p in range(2):
                                for c in (0, BK):
                                    pe.matmul(
                                        ps[:, p, c : c + BK], idt[:, :],
                                        o2t[:, lo + p, c : c + BK],
                                        start=True, stop=False,
                                    )
                            for p in range(2):
                                for c in (0, BK):
                                    pe.matmul(
                                        ps[:, p, c : c + BK], nidt[:, :],
                                        tA[:, lo + p, c : c + BK],
                                        start=False, stop=False,
                                    )
                            for p in range(2):
                                pe.matmul(
                                    ps[:, p, 1:BK], idt[:, :],
                                    tF[:, lo + p, 0 : BK - 1],
                                    start=False, stop=False,
                                )
                                pe.matmul(
                                    ps[:, p, BK:W], idt[:, :],
                                    tF[:, lo + p, BK - 1 : W - 1],
                                    start=False, stop=False,
                                )
                            for p in range(2):
                                for c in (0, BK):
                                    pe.matmul(
                                        ps[:, p, c : c + BK], idt[:, :],
                                        tH[:, lo + p, c : c + BK],
                                        start=False, stop=True,
                                    )
                            if not last:
                                act.activation(
                                    t_(lo, hi), ps[:, :, :], AF.Tanh, scale=0.5
                                )
                            else:
                                act.activation(t_(lo, hi), ps[:, :, :], AF.Copy)
                                nc.sync.dma_start(
                                    out=vu(out_d)[:, lo:hi, :], in_=t_(lo, hi)
                                )

    nc.compile()
    return nc


def kernel(o, vector_field, nabla_w, div_w):
    global LAST_RESULTS
    if "nc" not in _CACHE:
        _CACHE["nc"] = _build()
    nc = _CACHE["nc"]

    o2 = (2.0 * np.asarray(o, dtype=np.float32)[:, 0]).astype(np.float16)
    vf = np.asarray(vector_field, dtype=np.float32)
    s = np.float32(1.0 / np.sqrt(2.0))
    g0f = vf[:, :, 0] * s
    g1f = vf[:, :, 1] * s
    g0 = g0f.astype(np.float16)
    # the kernel reconstructs g1 = gs - g0 on-chip, so build gs to make
    # that reconstruction exact-ish in fp16: gs = fl16(g0) + fl16(g1)
    gs = (g0.astype(np.float32) + g1f.astype(np.float16).astype(np.float32)
          ).astype(np.float16)
    g1df = np.zeros_like(g1f)
    g1df[1:] = g1f[:-1]
    g1d = g1df.astype(np.float16)
    km = np.ascontiguousarray(-(gs[1023:1024, :].astype(np.float32)
                                - g0[1023:1024, :].astype(np.float32))
                              ).astype(np.float16)  # K at the t(1024)=-1 pad
    ident = np.eye(P, dtype=np.float16)

    in_maps = [
        {
            "o2": np.ascontiguousarray(o2[b]),
            "g0": g0,
            "gs": gs,
            "g1d": g1d,
            "km": km,
            "ident": ident,
            "nident": -ident,
        }
        for b in range(B)
    ]
    res = bass_utils.run_bass_kernel_spmd(nc, in_maps, core_ids=list(range(B)))
    LAST_RESULTS = res
    return np.stack([r["out"] for r in res.results]).astype(np.float32)


# revision 53
# speedup vs baseline: 1.0053x; 1.0053x over previous
"""Trainium2 Bass kernel for the Chambolle-Pock-style primal/dual stencil loop.

Math (per image, H=W=1024, EPS=0.5, TAU=0.5, 10 iterations):
    u = sigmoid(o/EPS); q = 0
    repeat 10x:
        q  = relu(q - TAU*(vf1*Dy(u) + vf0*Dx(u)))   # forward diffs, zero pad
        Tq = BDy(vf1*q) + BDx(vf0*q)                  # backward diffs, zero pad
        u  = sigmoid((o - Tq)/EPS)
    return (o - Tq)/EPS

Rescaled (qh = 2*sqrt(2)*q, g = vf/sqrt(2), o2 = 2*o, t = tanh(s/2) with
s = 2(o - Tq); u-padding 0 becomes t-padding -1) and with every y-shifted
product rewritten through a host-preshifted field (gs = g0+g1, g1d(y) =
g1(y-1)) so each product depends on exactly one tanh/relu quarter:
    K = g1d*t; H = g1*qh                  # then B(y)=g1(y)*t(y+1) = K(y+1)
    dual:   qh = relu(qh + gs*t - K(y+1) - g0*t(x+1))
    primal: s  = o2 - gs*qh + H(y-1) + (g0*qh)(x-1)
    t = tanh(s/2); output = s of the last iteration.

State and products are fp16 (DVE tensor_tensor runs at 2 elem/cyc for packed
2-byte operands; bf16 fails the 2e-2 gate because the relu makes isolated
pixels chaotic under per-step rounding noise — measured rel-L2 ~1e-2 for
fp16 vs ~3e-2 for bf16).  The primal accumulation happens in fp32 PSUM.

Engine split (four compute engines + DMA queues, all busy):
  - DVE: the A/K/H/D products, one quarter of F', and quarter q1's dual
    accumulations, all 4B-aligned fp16 tensor_tensor at 2 elem/cycle.
  - GPSIMD: the 2B-misaligned product C~ = g0*t(x+1) (GPSIMD is
    alignment-indifferent) and one quarter of F' = g0*qh.
  - PE (tensor engine): both chains as accumulating +-identity matmuls
    into fp32 PSUM, one 512-column matmul per PSUM bank:
    dual  psum = I*qh + I*A - I*K(y+1) - I*C~   (3 of 4 quarters)
    primal psum = I*o2 - I*(gs*qh) + I*F'(x-1) + I*H(y-1),
    the x-1 shift expressed directly via column-offset PSUM writes (s(0)
    keeps no F term).  Group heads (qh / o2) have no in-iteration deps so
    the PE never stalls at a bank's start matmul.
  - Act: relu and tanh (PSUM -> SBUF fp16), one quarter at a time; on the
    last iteration it instead copies s from PSUM for the output DMA.
  - DMA: the K/H boundary rows move between partitions via tiny SBUF->SBUF
    copies whose consumers sit half an iteration away.

Layout: image row y = 8*p + i -> partition p (0..127), plane i (0..7).
Everything operates at quarter (2-plane) granularity in fixed order
(1,2,3,0); per iteration the DVE does ~45 plane-passes at ~0.56us, GPSIMD
10 at 2.05us, PE 112 x 512-col matmuls (~24us), Act 16 quarter-activations.
Working set (~196KB/partition) is SBUF resident: HBM traffic is one 8MB
fp16 load + 2MB store per core.

Sharding: pure data parallel, one image per NeuronCore (B=8 over 8 cores),
g-fields broadcast.
"""

import numpy as np

import concourse.bacc as bacc
import concourse.mybir as mybir
from concourse.tile import TileContext
from concourse import bass_utils

F16 = mybir.dt.float16
F32 = mybir.dt.float32
AF = mybir.ActivationFunctionType

B, H, W = 8, 1024, 1024
P = 128          # SBUF partitions
NP = H // P      # planes per partition = 8
WG = W + 2       # t-plane width incl. guard column (even, keeps 4B align)
MAXITER = 10
QORD = ((2, 4), (4, 6), (6, 8), (0, 2))   # quarter order 1,2,3,0
BK = 512         # PSUM bank = 512 fp32 = one matmul's max output

_CACHE = {}
LAST_RESULTS = None  # BassKernelResults of the most recent run (for test.py)


def _build(reps=1):
    """Build the Bass program.  reps>1 repeats the whole computation (state
    re-initialized each rep, same output) — used only for wall-clock timing
    of the HW kernel when no NTFF profiling is available."""
    nc = bacc.Bacc("TRN2", target_bir_lowering=False, debug=False)

    o2_d = nc.dram_tensor("o2", [H, W], F16, kind="ExternalInput").ap()
    g0_d = nc.dram_tensor("g0", [H, W], F16, kind="ExternalInput").ap()
    gs_d = nc.dram_tensor("gs", [H, W], F16, kind="ExternalInput").ap()
    g1d_d = nc.dram_tensor("g1d", [H, W], F16, kind="ExternalInput").ap()
    km_d = nc.dram_tensor("km", [1, W], F16, kind="ExternalInput").ap()
    id_d = nc.dram_tensor("ident", [P, P], F16, kind="ExternalInput").ap()
    nid_d = nc.dram_tensor("nident", [P, P], F16, kind="ExternalInput").ap()
    out_d = nc.dram_tensor("out", [H, W], F16, kind="ExternalOutput").ap()

    # (H, W) -> (p, i, x) with y = 8*p + i
    def vu(ap):
        return ap.rearrange("(p i) x -> p i x", i=NP)

    v = nc.vector
    gp = nc.gpsimd
    act = nc.scalar
    pe = nc.tensor

    with TileContext(nc) as tc:
        with tc.tile_pool(name="main", bufs=1) as pool:
            o2t = pool.tile([P, NP, W], F16)
            g0t = pool.tile([P, NP, W], F16)
            g1t = pool.tile([P, NP, W], F16)
            gst = pool.tile([P, NP, W], F16)
            g1dt = pool.tile([P, NP, W], F16)
            # t state: planes 0..7, col W = -1 guard for x+1 reads
            sut = pool.tile([P, NP, WG], F16)
            qht = pool.tile([P, NP, W], F16)
            tA = pool.tile([P, NP, W], F16)
            # K = g1d*t at planes 0..7; plane 8 = K[8p+8] boundary
            # (partition 127: constant -g1[1023] = K at the t=-1 pad row)
            tK = pool.tile([P, NP + 1, W], F16)
            # H = g1*qh at planes 1..8; plane 0 = H[8p-1] boundary
            # (partition 0: zero pad)
            tH = pool.tile([P, NP + 1, W], F16)
            tC = pool.tile([P, NP, W], F16)
            tF = pool.tile([P, NP, W], F16)
            idt = pool.tile([P, P], F16)
            nidt = pool.tile([P, P], F16)

            def t_(lo, hi):     # t rows 8p+lo..8p+hi-1
                return sut[:, lo:hi, 0:W]

            def trt(lo, hi):    # t(x+1) (col W = -1 guard)
                return sut[:, lo:hi, 1 : W + 1]

            def bv(lo, hi):     # B(y) = K(y+1) (plane 8 = boundary)
                return tK[:, lo + 1 : hi + 1, :]

            def qh_(lo, hi):
                return qht[:, lo:hi, :]

            def sl(tile, lo, hi):
                return tile[:, lo:hi, :]

            # --- setup ---
            # Only guard regions need init: everything else is written
            # before its first read.  Loads are HBM-bandwidth serial
            # (~1.6us per field quarter), so interleave in need order;
            # g1 = gs - g0 is derived on-chip to cut the load volume.
            v.memset(sut[:, :, W:WG], -1.0)        # x+1 guard column = -1
            v.memset(tH[0:1, 0, :], 0.0)           # H[-1] pad row = 0
            for i, (lo, hi) in enumerate(QORD):
                nc.sync.dma_start(
                    out=o2t[:, lo:hi, :], in_=vu(o2_d)[:, lo:hi, :]
                )
                nc.sync.dma_start(
                    out=g0t[:, lo:hi, :], in_=vu(g0_d)[:, lo:hi, :]
                )
                nc.sync.dma_start(
                    out=gst[:, lo:hi, :], in_=vu(gs_d)[:, lo:hi, :]
                )
                nc.sync.dma_start(
                    out=g1dt[:, lo:hi, :], in_=vu(g1d_d)[:, lo:hi, :]
                )
                if i == 0:
                    # small constants after the first field quarter: their
                    # consumers (PE groups, dual q3) run mid-iteration-0,
                    # and issuing them first would delay the o2/tanh start
                    nc.sync.dma_start(out=tK[P - 1 : P, NP, :], in_=km_d)
                    nc.sync.dma_start(out=idt[:, :], in_=id_d)
                    nc.sync.dma_start(out=nidt[:, :], in_=nid_d)

            def dma_kshift():
                # tK[p, 8] = K[8p+8] = tK[p+1, 0]; partition 127 keeps km
                nc.sync.dma_start(
                    out=tK[0 : P - 1, NP, :], in_=tK[1:P, 0, :]
                )

            def dma_hshift():
                # tH[p, 0] = H[8p-1] = tH[p-1, 8]; partition 0 stays 0
                nc.sync.dma_start(
                    out=tH[1:P, 0, :], in_=tH[0 : P - 1, NP, :]
                )

            with tc.tile_pool(name="ps", bufs=2, space="PSUM") as pp:

                def alloc_ps():
                    # single allocation site -> one rotating pair of 4-bank
                    # PSUM buffers shared by the dual and primal chains
                    return pp.tile([P, 2, W], F32, name="ps")

                for _rep in range(reps):
                    for lo, hi in QORD:
                        act.activation(
                            t_(lo, hi), sl(o2t, lo, hi), AF.Tanh, scale=0.5
                        )

                    for it in range(MAXITER):
                        first = it == 0
                        last = it == MAXITER - 1
                        # --- dual products (each needs one tanh quarter) ---
                        # GPSIMD: C~ = g0*t(x+1) (misaligned read is free here)
                        for lo, hi in QORD:
                            gp.tensor_mul(
                                sl(tC, lo, hi), sl(g0t, lo, hi), trt(lo, hi)
                            )
                        # DVE: A = gs*t, K = g1d*t; g1 = gs - g0 (iter 0 only)
                        for lo, hi in QORD:
                            v.tensor_mul(sl(tA, lo, hi), sl(gst, lo, hi), t_(lo, hi))
                            v.tensor_mul(sl(tK, lo, hi), sl(g1dt, lo, hi), t_(lo, hi))
                            if first:
                                v.tensor_sub(
                                    sl(g1t, lo, hi), sl(gst, lo, hi), sl(g0t, lo, hi)
                                )
                        dma_kshift()
                        # --- dual chain: qh = relu(qh + A - K(y+1) - C~).
                        # Quarter q0 (processed last) runs as plain DVE
                        # accumulations — DVE has slack right after its
                        # product phase and q0's B-view needs no boundary
                        # row; the other quarters run on PE as +-identity
                        # matmuls into fp32 PSUM with relu reading PSUM on
                        # Act.  The PE group head (qh) has no in-iteration
                        # deps, so the PE never stalls at a start matmul.
                        for lo, hi in QORD:
                            if lo == 2:
                                if first:
                                    v.tensor_sub(
                                        qh_(lo, hi), sl(tA, lo, hi), bv(lo, hi)
                                    )
                                else:
                                    v.tensor_add(
                                        qh_(lo, hi), qh_(lo, hi), sl(tA, lo, hi)
                                    )
                                    v.tensor_sub(
                                        qh_(lo, hi), qh_(lo, hi), bv(lo, hi)
                                    )
                                v.tensor_sub(qh_(lo, hi), qh_(lo, hi), sl(tC, lo, hi))
                                act.activation(qh_(lo, hi), qh_(lo, hi), AF.Relu)
                                continue
                            ps = alloc_ps()
                            srcs = [] if first else [(idt, qht, lo)]
                            srcs += [(idt, tA, lo), (nidt, tK, lo + 1), (nidt, tC, lo)]
                            for i_s, (w, tile, plo) in enumerate(srcs):
                                for p in range(2):
                                    for c in (0, BK):
                                        pe.matmul(
                                            ps[:, p, c : c + BK], w[:, :],
                                            tile[:, plo + p, c : c + BK],
                                            start=(i_s == 0),
                                            stop=(i_s == len(srcs) - 1),
                                        )
                            act.activation(qh_(lo, hi), ps[:, :, :], AF.Relu)
                        # --- primal products (each needs one relu quarter);
                        # they must ALL precede the PE loop in program order
                        # because the E-view of quarter q reads H rows from
                        # quarter q-1 (including the wrap onto q0, last) ---
                        for lo, hi in QORD:
                            # F' = g0*qh: quarters q1/q2 on GPSIMD, rest DVE
                            if lo in (2,):
                                gp.tensor_mul(
                                    sl(tF, lo, hi), sl(g0t, lo, hi), qh_(lo, hi)
                                )
                            else:
                                v.tensor_mul(
                                    sl(tF, lo, hi), sl(g0t, lo, hi), qh_(lo, hi)
                                )
                            # H = g1*qh (planes 1..8 of tH), D = gs*qh
                            v.tensor_mul(
                                tH[:, lo + 1 : hi + 1, :], sl(g1t, lo, hi),
                                qh_(lo, hi),
                            )
                            if (lo, hi) == (6, 8):
                                dma_hshift()
                            v.tensor_mul(sl(tA, lo, hi), sl(gst, lo, hi), qh_(lo, hi))
                        # --- primal accumulation on PE: s = o2 - D + F'(x-1)
                        # + H(y-1), one matmul per PSUM bank, E-view last so
                        # only the group tail waits on the wrap H quarter;
                        # accumulation groups are tracked per 2KB bank, so
                        # each bank's final matmul carries stop=True ---
                        for lo, hi in QORD:
                            ps = alloc_ps()
                            for p in range(2):
                                for c in (0, BK):
                                    pe.matmul(
                                        ps[:, p, c : c + BK], idt[:, :],
                                        o2t[:, lo + p, c : c + BK],
                                        start=True, stop=False,
                                    )
                            for p in range(2):
                                for c in (0, BK):
                                    pe.matmul(
                                        ps[:, p, c : c + BK], nidt[:, :],
                                        tA[:, lo + p, c : c + BK],
                                        start=False, stop=False,
                                    )
                            for p in range(2):
                                pe.matmul(
                                    ps[:, p, 1:BK], idt[:, :],
                                    tF[:, lo + p, 0 : BK - 1],
                                    start=False, stop=False,
                                )
                                pe.matmul(
                                    ps[:, p, BK:W], idt[:, :],
                                    tF[:, lo + p, BK - 1 : W - 1],
                                    start=False, stop=False,
                                )
                            for p in range(2):
                                for c in (0, BK):
                                    pe.matmul(
                                        ps[:, p, c : c + BK], idt[:, :],
                                        tH[:, lo + p, c : c + BK],
                                        start=False, stop=True,
                                    )
                            if not last:
                                act.activation(
                                    t_(lo, hi), ps[:, :, :], AF.Tanh, scale=0.5
                                )
                            else:
                                act.activation(t_(lo, hi), ps[:, :, :], AF.Copy)
                                nc.sync.dma_start(
                                    out=vu(out_d)[:, lo:hi, :], in_=t_(lo, hi)
                                )

    nc.compile()
    return nc


def kernel(o, vector_field, nabla_w, div_w):
    global LAST_RESULTS
    if "nc" not in _CACHE:
        _CACHE["nc"] = _build()
    nc = _CACHE["nc"]

    o2 = (2.0 * np.asarray(o, dtype=np.float32)[:, 0]).astype(np.float16)
    vf = np.asarray(vector_field, dtype=np.float32)
    s = np.float32(1.0 / np.sqrt(2.0))
    g0f = vf[:, :, 0] * s
    g1f = vf[:, :, 1] * s
    g0 = g0f.astype(np.float16)
    # the kernel reconstructs g1 = gs - g0 on-chip, so build gs to make
    # that reconstruction exact-ish in fp16: gs = fl16(g0) + fl16(g1)
    gs = (g0.astype(np.float32) + g1f.astype(np.float16).astype(np.float32)
          ).astype(np.float16)
    g1df = np.zeros_like(g1f)
    g1df[1:] = g1f[:-1]
    g1d = g1df.astype(np.float16)
    km = np.ascontiguousarray(-(gs[1023:1024, :].astype(np.float32)
                                - g0[1023:1024, :].astype(np.float32))
                              ).astype(np.float16)  # K at the t(1024)=-1 pad
    ident = np.eye(P, dtype=np.float16)

    in_maps = [
        {
            "o2": np.ascontiguousarray(o2[b]),
            "g0": g0,
            "gs": gs,
            "g1d": g1d,
            "km": km,
            "ident": ident,
            "nident": -ident,
        }
        for b in range(B)
    ]
    res = bass_utils.run_bass_kernel_spmd(nc, in_maps, core_ids=list(range(B)))
    LAST_RESULTS = res
    return np.stack([r["out"] for r in res.results]).astype(np.float32)
